# revision 9
# baseline (speedup 1.0000x reference)
"""Trainium2 kernel for nn_COSSIMMLP (gnn_message_passing).

reference semantics:
    src = prop_state[b, mask[...,0]]; dst = prop_state[b, mask[...,1]]
    vals = sigmoid(cossim(src, dst))          # [B, E]
    adj[b, i, j] = vals; adj[b, j, i] = vals  # dense [B, N, N]

Every scatter write at position (r, c) carries the identical value
sigmoid(cos(s_r, s_c)), so the output is exactly

    adj = sigmoid(S_hat @ S_hat.T + Madd),  Madd = 0 at edge positions,
                                                   -240 elsewhere

with S_hat the eps-clamp-normalized rows.  sigmoid(x - 240) underflows to 0 in
f32, so non-edges are (numerically exact) zero.

Implementation highlights (167 us first-working -> this version):
  * gram matmul in fp8 DoubleRow perf mode (K=256 in one pass)
  * additive mask shipped as 1 bit/entry, expanded on the vector engine:
    one u16 tensor_scalar (shift + AND against 0x4040) per 512-column
    bit-plane yields bytes {0x00, 0x40} = fp8 {0, 2.0}; an identity scaled
    by -120 folds them into PSUM (masked entries get -240 before sigmoid).
  * f16 output tile + f16 HBM store (host widens to f32)
  * f16 prop input (host narrows; norm math still f32 on device)
  * norms via one fused DVE tensor_tensor_reduce per node tile
  * the 8.4M-element sigmoid paces phase B: PSUM is split 2x[128,2048] and
    the transpose staging borrows the same pool buffers; ACT tables are
    warmed off the critical path (dummy Sqrt at t0, dummy Sigmoid after the
    last norm sqrt)

Sharding: 8 cores = 4 batches x 2 row-halves.  Each core computes a
[2048, 4096] slab of one batch's adjacency.  Per-core node order is rolled
by the row offset so that a single SPMD program serves all cores; the host
un-rolls output columns.
"""

import numpy as np
import ml_dtypes

B, N, D, E = 4, 4096, 256, 131072
NH = N // 2          # rows per core
P = 128              # partitions
NT = N // P          # 32 node tiles
MT = NH // P         # 16 row tiles per core
GRP = 4              # node tiles per phase-A group
EPS = 1e-8

_prog = None


def _build_program():
    import concourse.tile as tile
    from concourse import bacc, mybir
    from concourse.masks import make_identity

    f32 = mybir.dt.float32
    f16 = mybir.dt.float16
    fp8 = mybir.dt.float8e4
    u16 = mybir.dt.uint16
    ACT = mybir.ActivationFunctionType
    ALU = mybir.AluOpType
    MM = mybir.MatmulPerfMode

    nc = bacc.Bacc("TRN2", target_bir_lowering=False, debug=False)
    s_in = nc.dram_tensor("s16", [N, D], f16, kind="ExternalInput")
    b_in = nc.dram_tensor("bits", [NH, N // 16], u16, kind="ExternalInput")
    out = nc.dram_tensor("out", [NH, N], f16, kind="ExternalOutput")

    with tile.TileContext(nc) as tc:
        with tc.tile_pool(name="const", bufs=1) as cpool:
            ident16 = cpool.tile([P, P], f16)
            make_identity(nc, ident16[:])
            identm = cpool.tile([P, P], fp8)
            make_identity(nc, identm[:])
            # fold identity scaled by -120: mask bytes are fp8 2.0 -> adds -240
            nc.vector.tensor_scalar_mul(out=identm[:], in0=identm[:], scalar1=-120.0)
            # preload the sqrt ACT table while DMAs are in flight
            warm = cpool.tile([P, 1], f32)
            nc.scalar.activation(out=warm[:], in_=ident16[:, 0:1], func=ACT.Sqrt)
            # S_hat.T in fp8, D split into 2 chunks paired for DoubleRow
            stp = cpool.tile([P, 2, N], fp8)
            # all mask bits resident: row m*128+p -> bitsb[p, m, :]
            bitsb = cpool.tile([P, MT, N // 16], u16)

            with (
                tc.tile_pool(name="prep", bufs=1) as prep,
                tc.tile_pool(name="prep_sc", bufs=2) as prep_sc,
                tc.tile_pool(name="mrow", bufs=3) as mrow,
                tc.tile_pool(name="outp", bufs=4) as outp,
                tc.tile_pool(name="mmps", bufs=2, space="PSUM") as mmps,
            ):
                s_sb = prep.tile([P, NT, D], f16)
                sh16 = prep.tile([P, NT, D], f16)
                stats = prep.tile([P, NT, 6], f32)
                s_r = s_in.rearrange("(t p) d -> p t d", p=P)

                # ---- phase A: per-group load -> norms -> scale ----
                for grp in range(NT // GRP):
                    t0 = grp * GRP
                    nc.sync.dma_start(
                        out=s_sb[:, t0 : t0 + GRP, :], in_=s_r[:, t0 : t0 + GRP, :]
                    )
                    if grp == 0:
                        # mask bits ride the other HWDGE queue
                        nc.scalar.dma_start(
                            out=bitsb[:], in_=b_in.rearrange("(m p) c -> p m c", p=P)
                        )
                    for i in range(GRP):
                        nc.vector.bn_stats(
                            out=stats[:, t0 + i, :], in_=s_sb[:, t0 + i, :]
                        )
                    sl = slice(t0, t0 + GRP)
                    me2 = prep_sc.tile([P, GRP], f32, tag="me2")
                    nc.vector.tensor_tensor(
                        out=me2[:], in0=stats[:, sl, 1], in1=stats[:, sl, 1], op=ALU.mult
                    )
                    mo2 = prep_sc.tile([P, GRP], f32, tag="mo2")
                    nc.vector.tensor_tensor(
                        out=mo2[:], in0=stats[:, sl, 4], in1=stats[:, sl, 4], op=ALU.mult
                    )
                    nc.vector.tensor_tensor(
                        out=me2[:], in0=me2[:], in1=mo2[:], op=ALU.add
                    )
                    nsq = prep_sc.tile([P, GRP], f32, tag="nsq")
                    nc.vector.tensor_tensor(
                        out=nsq[:], in0=stats[:, sl, 2], in1=stats[:, sl, 5], op=ALU.add
                    )
                    nc.vector.scalar_tensor_tensor(
                        out=nsq[:], in0=me2[:], scalar=float(D // 2), in1=nsq[:],
                        op0=ALU.mult, op1=ALU.add,
                    )
                    nrm = prep_sc.tile([P, GRP], f32, tag="nrm")
                    nc.scalar.activation(out=nrm[:], in_=nsq[:], func=ACT.Sqrt)
                    inv = prep_sc.tile([P, GRP], f32, tag="inv")
                    nc.vector.reciprocal(out=inv[:], in_=nrm[:])
                    for i in range(GRP):
                        nc.vector.tensor_scalar_mul(
                            out=sh16[:, t0 + i, :],
                            in0=s_sb[:, t0 + i, :],
                            scalar1=inv[:, i : i + 1],
                        )
                # warm the sigmoid table while the PE transposes run
                nc.scalar.activation(out=warm[:], in_=ident16[:, 0:1], func=ACT.Sigmoid)

                def emit_transposes(grp):
                    t0 = grp * GRP
                    for i in range(2):
                        tps = mmps.tile([P, GRP, P], f16, tag="ps")
                        for tt in range(GRP):
                            nc.tensor.transpose(
                                tps[:, tt, :],
                                sh16[:, t0 + tt, i * P : (i + 1) * P],
                                ident16[:],
                            )
                        nc.vector.tensor_copy(
                            out=stp[:, i, t0 * P : (t0 + GRP) * P], in_=tps[:]
                        )

                def emit_expand(m):
                    madd = mrow.tile([P, N // 2], u16, tag="madd")
                    for k in range(8):
                        dst = madd[:, k * (N // 16) : (k + 1) * (N // 16)]
                        if k == 6:
                            nc.vector.tensor_scalar(
                                out=dst, in0=bitsb[:, m, :], scalar1=0x4040,
                                scalar2=None, op0=ALU.bitwise_and,
                            )
                        elif k < 6:
                            nc.vector.tensor_scalar(
                                out=dst, in0=bitsb[:, m, :], scalar1=6 - k,
                                scalar2=0x4040, op0=ALU.logical_shift_left,
                                op1=ALU.bitwise_and,
                            )
                        else:
                            nc.vector.tensor_scalar(
                                out=dst, in0=bitsb[:, m, :], scalar1=k - 6,
                                scalar2=0x4040, op0=ALU.logical_shift_right,
                                op1=ALU.bitwise_and,
                            )
                    return madd

                def emit_half(m, g, madd):
                    lhsT = stp[:, :, m * P : (m + 1) * P]
                    ps = mmps.tile([P, 2048], f32, tag="ps")
                    for q in range(4):
                        c0 = g * 2048 + q * 512
                        nc.tensor.matmul(
                            ps[:, q * 512 : (q + 1) * 512],
                            lhsT=lhsT,
                            rhs=stp[:, :, c0 : c0 + 512],
                            start=True,
                            stop=False,
                            perf_mode=MM.DoubleRow,
                        )
                    for q in range(4):
                        c0 = g * 2048 + q * 512
                        nc.tensor.matmul(
                            ps[:, q * 512 : (q + 1) * 512],
                            lhsT=identm[:],
                            rhs=madd[:, c0 // 2 : c0 // 2 + 256].bitcast(fp8),
                            start=False,
                            stop=True,
                        )
                    ot = outp.tile([P, 2048], f16, tag="ot")
                    nc.scalar.activation(out=ot[:], in_=ps[:], func=ACT.Sigmoid)
                    nc.sync.dma_start(
                        out=out[m * P : (m + 1) * P, g * 2048 : (g + 1) * 2048],
                        in_=ot[:],
                    )

                # ---- phase B interleaved with second-half transposes ----
                for g in range(4):
                    emit_transposes(g)
                madds = {m: emit_expand(m) for m in range(3)}
                for m in range(3):
                    emit_half(m, 0, madds[m])
                for g in range(4, 8):
                    emit_transposes(g)
                for m in range(3):
                    emit_half(m, 1, madds[m])
                for m in range(3, MT):
                    madd = emit_expand(m)
                    emit_half(m, 0, madd)
                    emit_half(m, 1, madd)

    nc.compile()
    return nc


def _host_prep(prop_state, mask):
    prop = np.asarray(prop_state)
    mk = np.asarray(mask)
    i = mk[..., 0].astype(np.int64)
    j = mk[..., 1].astype(np.int64)
    # dense edge indicator per batch, as flat bool
    edge = np.zeros((B, N * N), dtype=bool)
    for b in range(B):
        edge[b][i[b] * N + j[b]] = True
        edge[b][j[b] * N + i[b]] = True
    edge = edge.reshape(B, N, N)
    prop16 = prop.astype(np.float16)

    in_maps = []
    for c in range(8):
        b, h = divmod(c, 2)
        r = h * NH
        s_roll = prop16[b] if r == 0 else np.roll(prop16[b], -r, axis=0)
        ne = ~edge[b][r : r + NH]
        if r:
            ne = np.roll(ne, -r, axis=1)
        # byte c bit k = nonedge(row, k*512 + c); u16 = little-endian byte pair
        bits = np.packbits(
            ne.reshape(NH, 8, N // 8), axis=1, bitorder="little"
        ).reshape(NH, N // 8)
        in_maps.append(
            {
                "s16": np.ascontiguousarray(s_roll),
                "bits": np.ascontiguousarray(bits).view("<u2"),
            }
        )
    return in_maps


def _assemble(results):
    outf = np.empty((B, N, N), dtype=np.float32)
    for c in range(8):
        b, h = divmod(c, 2)
        r = h * NH
        o = results[c]["out"].astype(np.float32)
        outf[b, r : r + NH, :] = o if r == 0 else np.roll(o, r, axis=1)
    return outf


def kernel(prop_state, mask):
    from concourse.bass_utils import run_bass_kernel_spmd

    global _prog
    if _prog is None:
        _prog = _build_program()
    in_maps = _host_prep(prop_state, mask)
    res = run_bass_kernel_spmd(_prog, in_maps, core_ids=list(range(8)))
    return _assemble(res.results)
